# revision 1
# baseline (speedup 1.0000x reference)
"""FAIM head kernel for Trainium2 (8 NeuronCores, SPMD over class shards).

Computes out[b,c] = -scale * (sqrt((x_b-mu_c)^T Sigma (x_b-mu_c) + eps)
                              + lmbda * sqrt((beta.(x_b-mu_c))^2 + eps))
with Sigma = tril(L) @ tril(L)^T + eps*I.

Algebraic decomposition (validated to ~1e-6 rel err vs the naive reference):
with Lt = tril(L), YT = Lt^T x^T [D,B], MT = Lt^T mu^T [D,C]:
  quad[b,c] = a[b] + g[c] - 2*cross[b,c]
  a[b]      = |Y_b|^2 + eps*|x_b|^2     (diagonal of the Y gram matrix)
  g[c]      = |M_c|^2 + eps*|mu_c|^2
  cross     = Y M^T + eps * x mu^T
  beta_dot  = (x beta)[b] - (mu beta)[c]
This turns the reference's 268 GFLOP einsum into ~3 GFLOP of matmuls.

All matmul operands are viewed as float32r (same fp32 bits; fast PE path at
moving-dim >= 256, which is why the [xT|muT] combined width is padded
253 -> 256 with zeros).

Sharding: classes C=1000 split 125 per core; x/L/beta replicated.
"""

import numpy as np

try:
    import concourse.bass as bass
except ImportError:  # pragma: no cover
    import sys

    sys.path.insert(0, "/opt/trn_rl_repo")
    import concourse.bass as bass

import concourse.bacc as bacc
import concourse.mybir as mybir
import concourse.tile as tile
from concourse.bass_utils import run_bass_kernel_spmd
from concourse.masks import make_identity, make_lower_triangular

F32 = mybir.dt.float32
F32R = mybir.dt.float32r
EPS = 1e-6
B, C, D = 128, 1000, 1024
NCORES = 8
CS = C // NCORES  # 125 classes per core
ND = D // 128  # 8 chunks of 128 along D
W = 128 + CS  # 253 used cols of the [xT | muT] block; padded to 256

_cached_nc = None


def _build(rep=1):
    # rep>1 unrolls the whole body rep times — used only by test.py to
    # measure marginal per-iteration device time (tunnel dispatch overhead
    # cancels); kernel() always uses rep=1.
    nc = bacc.Bacc(
        "TRN2", target_bir_lowering=False, debug=False, num_devices=NCORES
    )
    # xmu: x stacked with the (zero-padded) mu shard -> one DMA stream.
    # bpack: betaT chunks | lmbda bcast | scale bcast -> one tiny DMA.
    xmu_d = nc.dram_tensor("xmu", [2, 128, D], F32R, kind="ExternalInput")
    L_d = nc.dram_tensor("L", [D, D], F32R, kind="ExternalInput")
    bp_d = nc.dram_tensor("bpack", [128, ND + 2], F32, kind="ExternalInput")
    out_d = nc.dram_tensor("out", [B, CS], F32, kind="ExternalOutput")

    with tile.TileContext(nc) as tc:
        with (
            tc.tile_pool(name="const", bufs=1) as const,
            tc.tile_pool(name="data", bufs=1) as data,
            tc.tile_pool(name="epi", bufs=1) as epi,
            tc.tile_pool(name="pst", bufs=2, space="PSUM") as pst,
            tc.tile_pool(name="psy", bufs=4, space="PSUM") as psy,
            tc.tile_pool(name="acc", bufs=1, space="PSUM") as acc,
        ):
            for _r_i in range(rep):
                # ------- constants (small DMAs ride the ACT queue) ---------
                ident = const.tile([128, 128], F32)
                make_identity(nc, ident)
                trilm = const.tile([128, 128], F32)
                make_lower_triangular(nc, trilm, val=1.0, diag=True)
                identr = const.tile([128, 128], F32R)
                nc.vector.tensor_copy(out=identr, in_=ident)
                ident2 = const.tile([128, 128], F32)
                nc.vector.tensor_scalar_mul(out=ident2, in0=ident, scalar1=2.0)
                onesf = const.tile([128, 128], F32)
                nc.vector.memset(onesf, 1.0)
                ones = const.tile([128, 128], F32R)
                nc.vector.tensor_copy(out=ones, in_=onesf)
                neghalf = const.tile([128, 128], F32R)
                nc.vector.tensor_scalar_mul(out=neghalf, in0=onesf, scalar1=-0.5)
                negepsh = const.tile([128, 128], F32R)
                nc.vector.tensor_scalar_mul(
                    out=negepsh, in0=onesf, scalar1=-EPS / 2.0
                )
                epsb = const.tile([128, 1], F32)
                nc.vector.memset(epsb, EPS)
                epscol = const.tile([128, 1], F32R)
                nc.vector.tensor_scalar_mul(
                    out=epscol, in0=onesf[:, 0:1], scalar1=EPS
                )

                bp_sb = const.tile([128, ND + 2], F32)
                nc.scalar.dma_start(out=bp_sb, in_=bp_d[:])
                beta_sb = bp_sb[:, 0:ND]
                nsc_sb = const.tile([128, 1], F32)
                nc.scalar.mul(out=nsc_sb, in_=bp_sb[:, ND + 1 : ND + 2], mul=-1.0)
                lmn_sb = const.tile([128, 1], F32)
                nc.vector.tensor_mul(
                    out=lmn_sb, in0=bp_sb[:, ND : ND + 1], in1=nsc_sb
                )

                # ------- input DMAs: per-d rounds, descending --------------
                # each round d ships x-chunk d, mu-chunk d, then L panel d so
                # the transpose/YMT pipeline tracks DMA arrival.
                xmu_sb = data.tile([128, 2, D], F32R)
                xmu_r = xmu_d[:].rearrange("n p c -> p n c")
                nc.sync.dma_start(
                    out=xmu_sb[:, :, 512:1024], in_=xmu_r[:, :, 512:1024]
                )
                nc.sync.dma_start(
                    out=xmu_sb[:, :, 0:512], in_=xmu_r[:, :, 0:512]
                )
                # L column-slabs: slab j holds L[j*128:, j*128:(j+1)*128]
                # rearranged [128, ND-j, 128] — exactly the lhsT blocks of
                # YMT group j. Ascending order: group j unblocks on one DMA,
                # and the last-arriving slab feeds the smallest group.
                Ls_sb = [None] * ND
                for j in range(ND):
                    slab = data.tile(
                        [128, ND - j, 128], F32R, name=f"Ls{j}", tag=f"Ls{j}"
                    )
                    nc.sync.dma_start(
                        out=slab,
                        in_=L_d[
                            j * 128 : D, j * 128 : (j + 1) * 128
                        ].rearrange("(n p) c -> p n c", p=128),
                    )
                    # mask the diagonal block (n=0) to lower-triangular
                    # (Pool engine: slow but otherwise idle)
                    nc.gpsimd.tensor_mul(
                        out=slab[:, 0, :], in0=slab[:, 0, :], in1=trilm
                    )
                    Ls_sb[j] = slab


                # ------- PE p-state warmup during the DMA window -----------
                for i in range(6):
                    pw = psy.tile(
                        [128, 256], F32, name=f"pwarm{i}", tag="py"
                    )
                    nc.tensor.matmul(
                        pw[:1, :128], lhsT=ones[:, 0:1], rhs=ones
                    )

                # beta broadcast tiles, built during the DMA window
                bcast = [None] * ND
                for d in range(ND):
                    bb = data.tile(
                        [128, 128], F32R, name=f"bcast{d}", tag=f"bcast{d}"
                    )
                    nc.scalar.activation(
                        out=bb, in_=ones,
                        func=mybir.ActivationFunctionType.Copy,
                        scale=beta_sb[:, d : d + 1],
                    )
                    bcast[d] = bb

                # persistent accumulators (opened early, closed at the tail)
                # pc cols 0:128  = Ygram - a_bcast/2   (diagonal = a/2)
                # pc cols 128:253 = cross - g/2        (so quad = a - 2*pc)
                pc = acc.tile([128, 256], F32, name="pcross", tag="pcross")
                pbb = acc.tile([128, 256], F32, name="pbb", tag="pbb")

                # ---- transposes (descending d, tracking DMA arrival) ------
                # xmuT[d] = [x^T_d | mu^T_d | 0pad]  [128, 256]
                # As soon as xmuT[d] exists, its eps-cross / eps-g / beta
                # matmuls are issued so the accumulations drain early.
                xmuT = [None] * ND
                xmuT2 = [None] * ND
                xTe = [None] * ND
                for k, d in enumerate(range(ND - 1, -1, -1)):
                    cols = slice(d * 128, (d + 1) * 128)
                    pt = pst.tile([128, 256], F32R, name=f"pt{d}", tag="pt")
                    nc.tensor.transpose(
                        pt[:, 0:128], xmu_sb[:, 0, cols], identr
                    )
                    nc.tensor.transpose(
                        pt[:, 128:256], xmu_sb[:, 1, cols], identr
                    )

                    # the copy is pt's only reader, so the pst slot frees
                    # fast and transposes stream; square/eps read the copy.
                    # mu pad rows are zero (host-side): full-width copies.
                    xm = data.tile(
                        [128, 256], F32R, name=f"xmuT{d}", tag=f"xmuT{d}"
                    )
                    nc.vector.tensor_copy(out=xm, in_=pt)
                    xm2 = data.tile(
                        [128, 256], F32R, name=f"xmuT2_{d}", tag=f"xmuT2_{d}"
                    )
                    nc.scalar.square(out=xm2, in_=xm)
                    xe = data.tile(
                        [128, 128], F32R, name=f"xTe{d}", tag=f"xTe{d}"
                    )
                    nc.gpsimd.tensor_scalar_mul(
                        out=xe, in0=xm[:, 0:128], scalar1=EPS
                    )
                    xmuT[d], xmuT2[d], xTe[d] = xm, xm2, xe

                    first = k == 0
                    last = k == ND - 1
                    nc.tensor.matmul(
                        pbb, lhsT=bcast[d], rhs=xm, start=first, stop=last
                    )
                    nc.tensor.matmul(
                        pc, lhsT=xe, rhs=xm, start=first, stop=False
                    )
                    nc.tensor.matmul(
                        pc, lhsT=negepsh, rhs=xm2, start=False, stop=False
                    )

                # ---- YMT[j] = [Lt^T x^T | Lt^T mu^T | 0] block-row j ------
                # within a group, d runs high->low so the matmul needing the
                # last-arriving L panel (d=j) comes last; cross/g matmuls
                # trail one j-group behind to overlap the copy+square.
                YMT = [None] * ND
                YMT2 = [None] * ND

                def trail(j):
                    nc.tensor.matmul(
                        pc,
                        lhsT=YMT[j][:, :128],
                        rhs=YMT[j],
                        start=False,
                        stop=False,
                    )
                    nc.tensor.matmul(
                        pc,
                        lhsT=neghalf,
                        rhs=YMT2[j],
                        start=False,
                        stop=(j == ND - 1),
                    )

                for j in range(ND):
                    py = psy.tile([128, 256], F32, name=f"py{j}", tag="py")
                    # d=j (the masked diagonal block) goes last so the Pool
                    # mask has slack; the rest descend to match transpose
                    # completion order.
                    ds = list(range(ND - 1, j, -1)) + [j]
                    for i, d in enumerate(ds):
                        nc.tensor.matmul(
                            py,
                            lhsT=Ls_sb[j][:, d - j, :],
                            rhs=xmuT[d],
                            start=(i == 0),
                            stop=(i == len(ds) - 1),
                        )
                    ym = data.tile(
                        [128, 256], F32R, name=f"YMT{j}", tag=f"YMT{j}"
                    )
                    nc.vector.tensor_copy(out=ym, in_=py)
                    ym2 = data.tile(
                        [128, 256], F32R, name=f"YMT2_{j}", tag=f"YMT2_{j}"
                    )
                    nc.scalar.square(out=ym2, in_=py)
                    YMT[j], YMT2[j] = ym, ym2
                for j in range(ND):
                    trail(j)

                # ---------------- epilogue ----------------
                # a[b]+eps, bx[b]: diagonals of the partition-broadcast grams
                atmp = epi.tile([128, 128], F32)
                nc.vector.tensor_mul(out=atmp, in0=pc[:, :128], in1=ident2)
                aeps_sb = epi.tile([128, 1], F32)
                nc.vector.tensor_reduce(
                    out=aeps_sb, in_=atmp, axis=mybir.AxisListType.X,
                    op=mybir.AluOpType.add,
                )
                nc.vector.tensor_scalar_add(
                    out=aeps_sb, in0=aeps_sb, scalar1=EPS
                )
                bxtmp = epi.tile([128, 128], F32)
                nc.vector.tensor_mul(out=bxtmp, in0=pbb[:, :128], in1=ident)
                bx_sb = epi.tile([128, 1], F32)
                nc.vector.tensor_reduce(
                    out=bx_sb, in_=bxtmp, axis=mybir.AxisListType.X,
                    op=mybir.AluOpType.add,
                )

                # directional side closes early (pbb stops in the transpose
                # phase): lam_dir_s = lmbda*(-scale)*sqrt((bmu-bx)^2 + eps)
                bd = epi.tile([128, CS], F32)
                nc.vector.tensor_scalar_sub(
                    out=bd, in0=pbb[:, 128:W], scalar1=bx_sb
                )
                bd2 = epi.tile([128, CS], F32)
                nc.vector.tensor_mul(out=bd2, in0=bd, in1=bd)
                dirr = epi.tile([128, CS], F32)
                nc.scalar.activation(
                    out=dirr, in_=bd2, func=mybir.ActivationFunctionType.Sqrt,
                    bias=epsb,
                )
                lam_dir = epi.tile([128, CS], F32)
                nc.vector.tensor_scalar_mul(
                    out=lam_dir, in0=dirr, scalar1=lmn_sb
                )

                # qa = g - 2*cross ; riem = sqrt(qa + (a + eps))
                qa = epi.tile([128, CS], F32)
                nc.vector.tensor_scalar_mul(
                    out=qa, in0=pc[:, 128:W], scalar1=-2.0
                )
                riem = epi.tile([128, CS], F32)
                nc.scalar.activation(
                    out=riem, in_=qa, func=mybir.ActivationFunctionType.Sqrt,
                    bias=aeps_sb,
                )

                # out = riem*(-scale) + lam_dir_s
                res = epi.tile([128, CS], F32)
                nc.vector.scalar_tensor_tensor(
                    out=res, in0=riem, scalar=nsc_sb, in1=lam_dir,
                    op0=mybir.AluOpType.mult, op1=mybir.AluOpType.add,
                )
                nc.sync.dma_start(out=out_d[:], in_=res)

    nc.compile()
    return nc


def kernel(x, mu, beta, L, lmbda, scale, **kwargs):
    global _cached_nc
    if _cached_nc is None:
        _cached_nc = _build()
    nc = _cached_nc

    x = np.asarray(x, dtype=np.float32)
    mu = np.asarray(mu, dtype=np.float32)
    L = np.ascontiguousarray(np.asarray(L, dtype=np.float32))
    bpack = np.zeros((128, ND + 2), dtype=np.float32)
    bpack[:, 0:ND] = np.asarray(beta, dtype=np.float32).reshape(ND, 128).T
    bpack[:, ND] = np.float32(lmbda)
    bpack[:, ND + 1] = np.float32(scale)

    in_maps = []
    for i in range(NCORES):
        xmu = np.zeros((2, 128, D), dtype=np.float32)
        xmu[0] = x
        xmu[1, :CS] = mu[i * CS : (i + 1) * CS]
        in_maps.append({"xmu": xmu, "L": L, "bpack": bpack})
    res = run_bass_kernel_spmd(nc, in_maps, core_ids=list(range(NCORES)))
    return np.concatenate(
        [res.results[i]["out"] for i in range(NCORES)], axis=1
    )

